# revision 76
# baseline (speedup 1.0000x reference)
"""Trainium2 Bass kernel for nn_DiscreteTernaryBlock (8-core tensor-parallel).

Transformer block: rmsnorm -> QKV(ternary) -> causal attn (+alpha*x_heads)
-> Wo -> residual -> rmsnorm -> gated MLP (ternary) -> residual.

Sharding (all static SPMD; core c of 8):
  - tokens: 16 tiles of PT=128; core c OWNS global tiles {c, 8+c} (for norms,
    residuals, Wo, final output)
  - heads:  core c computes heads {2c, 2c+1} of attention (column-shard of
    Wq/Wk/Wv) over ALL tokens
  - mlp:    core c owns dff slice [1024c:1024c+1024] (column-shard Wg/Wu,
    row-shard Wd) over ALL tokens
Collectives: AG(normed) + A2A(x_heads) -> attn -> A2A(head outs) -> Wo ->
  AG(normed2) -> MLP -> 2x chunked ReduceScatter(partial mlp).

All matmuls run in bf16 (ternary weights are exact in bf16), fp32 PSUM
accumulation, fp32 residual path.
"""

import os
import sys
import math
from dataclasses import dataclass

import numpy as np

sys.path.insert(0, "/opt/trn_rl_repo")

import ml_dtypes

import concourse.bass as bass
import concourse.mybir as mybir
import concourse.tile as tile
from concourse import bacc
from concourse import bass_utils

BF16 = mybir.dt.bfloat16
FP32 = mybir.dt.float32
F8 = mybir.dt.float8e4
DR = mybir.MatmulPerfMode.DoubleRow
AF = mybir.ActivationFunctionType
ALU = mybir.AluOpType


@dataclass(frozen=True)
class Cfg:
    NC: int = 8     # cores
    G: int = 2      # token ownership groups (tiles per core)
    PT: int = 128   # token tile (partition dim for token-major tiles)
    DH: int = 128   # head dim
    HL: int = 2     # heads per core
    DFFL: int = 1024  # dff per core
    PF: int = 128   # feature tile (contraction partition dim)
    NCH: int = 512  # token chunk (matmul moving free dim)

    @property
    def D(self):
        return self.NC * self.HL * self.DH

    @property
    def S(self):
        return self.NC * self.G * self.PT

    @property
    def H(self):
        return self.NC * self.HL

    @property
    def DFF(self):
        return self.NC * self.DFFL

    @property
    def KF(self):
        return self.D // self.PF  # feature K tiles

    @property
    def TPC(self):
        return self.NCH // self.PT  # token tiles per chunk

    @property
    def NCHUNKS(self):
        return self.S // self.NCH

    @property
    def MD(self):
        return self.DFFL // self.PF  # dff tiles (per core)

    @property
    def ND(self):
        return self.D // self.NCH  # out-feature chunks

    @property
    def TT(self):
        return self.S // self.PT  # total token tiles


FULL = Cfg()
SMALL = Cfg(PT=16, DH=32, DFFL=128, NCH=64)


def declare_io(nc, cfg: Cfg):
    c = cfg
    io = {}

    def inp(name, shape, dt):
        io[name] = nc.dram_tensor(name, list(shape), dt, kind="ExternalInput").ap()

    inp("x_own", [c.G * c.PT, c.D], FP32)
    inp("w_attn", [1, c.D], FP32)
    inp("w_mlp", [1, c.D], FP32)
    inp("wqT", [c.D, c.HL * c.DH], F8)
    inp("wkT", [c.D, c.HL * c.DH], F8)
    inp("wvT", [c.D, c.HL * c.DH], F8)
    inp("woT", [c.D, c.D], F8)
    inp("wgT", [c.D, c.DFFL], F8)
    inp("wuT", [c.D, c.DFFL], F8)
    inp("wdT", [c.DFFL, c.D], BF16)
    io["out_own"] = nc.dram_tensor(
        "out_own", [c.G * c.PT, c.D], FP32, kind="ExternalOutput"
    ).ap()
    return io


def build_block(tc, io, cfg: Cfg, sc):
    """Emit the whole block program. sc: dict of baked scalar constants.

    Two pipelined token-group streams: for each ownership group g the chain
    norm1 -> AG -> qkv -> attn -> A2A -> Wo -> norm2 -> AG -> mlp -> RS runs
    offset so every collective overlaps the other group's compute.
    Gathered buffers are token-major; feature-major SBUF tiles are produced
    by DMA-transpose on load. alpha*x_heads is folded into a second Wo
    matmul (wo2T = alpha-scaled WoT) over the core's own normed tokens.
    """
    c = cfg
    nc = tc.nc
    RG = [list(range(c.NC))]
    DSH = c.D // c.NC       # feature rows per a2a shard (= HL*DH)
    CPG = c.NCHUNKS // c.G  # token chunks per ownership group

    # ---------------- pools ----------------
    singles = tc.alloc_tile_pool(name="singles", bufs=1)
    psum = tc.alloc_tile_pool(name="psum", bufs=1, space="PSUM")
    dram = tc.alloc_tile_pool(name="dram", bufs=1, space="DRAM")
    resid = tc.alloc_tile_pool(name="resid", bufs=1)

    # ---------------- dram scratch (split by ownership group g) ----------------
    HD = c.D // 2  # feature-half rows: AGs split in two so consumers start
    # on the first half while the second is still in flight
    nrm_own = [dram.tile([c.D, c.PT], F8, name=f"nrm_own{g}", tag=f"nrm_own{g}")
               for g in range(c.G)]
    ag1 = [dram.tile([c.NC * c.D, c.PT], F8, addr_space="Shared",
                     name=f"ag1_{g}", tag=f"ag1_{g}") for g in range(c.G)]
    attn_in = [dram.tile([c.D, c.PT], F8, name=f"attn_in{g}", tag=f"attn_in{g}")
               for g in range(c.G)]
    attn_a2a = [dram.tile([c.D, c.PT], F8, name=f"attn_a2a{g}", tag=f"attn_a2a{g}")
                for g in range(c.G)]
    nrm2_own = [dram.tile([c.D, c.PT], F8, name=f"nrm2_own{g}", tag=f"nrm2_own{g}")
                for g in range(c.G)]
    ag2 = [dram.tile([c.NC * c.D, c.PT], F8, addr_space="Shared",
                     name=f"ag2_{g}", tag=f"ag2_{g}") for g in range(c.G)]
    # RS chunking per group: [(start_nd, n_nds), ...]; the LAST group gets a
    # small final chunk so the un-overlapped tail RS is short.
    RS_SPLITS = [[(0, 2), (2, 2)], [(0, 1), (1, 1), (2, 1), (3, 1)]]
    mlp_part = [[dram.tile([c.NC * c.PT, cnt * c.NCH], BF16,
                           name=f"mlp_part{g}_{p}", tag=f"mlp_part{g}_{p}")
                 for p, (_, cnt) in enumerate(RS_SPLITS[g])] for g in range(c.G)]
    rs_out = [[dram.tile([c.PT, cnt * c.NCH], BF16,
                         name=f"rs_out{g}_{p}", tag=f"rs_out{g}_{p}")
               for p, (_, cnt) in enumerate(RS_SPLITS[g])] for g in range(c.G)]



    # ---------------- constants ----------------
    ident_pf = singles.tile([c.PF, c.PF], BF16)
    from concourse.masks import make_identity

    make_identity(nc, ident_pf)
    # [PT, 2, DH] of ones: the row-sum matmul uses all DH output partitions
    # so the softmax denominator lands replicated across partitions (no
    # gpsimd partition_broadcast / single-lane reciprocal needed).
    ones_pair = singles.tile([c.PT, 2, c.DH], F8)
    nc.vector.memset(ones_pair, 1.0)
    # alpha per feature-row, laid out as [PF, KF] (feature f -> alpha[f // DH])
    alpha_cols = singles.tile([c.PF, c.KF], FP32)
    for h in range(c.H):
        f0 = h * c.DH
        kf0, p0 = f0 // c.PF, f0 % c.PF
        nc.vector.memset(alpha_cols[p0 : p0 + c.DH, kf0 : kf0 + 1],
                         sc["alpha"][h])
    eps_sb = singles.tile([c.PT, 1], FP32)
    nc.vector.memset(eps_sb, sc["eps"])
    wmb = singles.tile([c.PT, c.D], FP32)
    nc.sync.dma_start(out=wmb, in_=io["w_mlp"].to_broadcast((c.PT, c.D)))

    x_sb = []   # per-group fp32 [PT, D] input rows (kept for residual)
    x2_sb = []  # per-group fp32 [PT, D] post-attention residual

    # ---------------- helpers ----------------
    def rmsnorm_to_dram(pool, src_sb, w_bc, dst_dram, tag, xsq=None, out_dt=BF16):
        """token-major src [PT, D] fp32 -> normed bf16 -> PE transpose ->
        feature-major dram [D, PT] (dtype out_dt)."""
        if xsq is None:
            xsq = pool.tile([c.PT, c.D], FP32, name=f"xsq_{tag}",
                            tag="nrm_xsq", bufs=1)
            nc.vector.tensor_mul(xsq, src_sb, src_sb)
        ssum = pool.tile([c.PT, 1], FP32, name=f"ssum_{tag}", tag="nrm_ssum")
        nc.vector.tensor_reduce(ssum, xsq, axis=mybir.AxisListType.X, op=ALU.add)
        nc.scalar.activation(ssum, ssum, AF.Sqrt, bias=eps_sb, scale=1.0 / c.D)
        nc.vector.reciprocal(ssum, ssum)
        nrm = pool.tile([c.PT, c.D], BF16, name=f"nrm_{tag}", tag="nrm_bf", bufs=1)
        nc.vector.scalar_tensor_tensor(
            out=nrm, in0=src_sb, scalar=ssum, in1=w_bc, op0=ALU.mult, op1=ALU.mult
        )
        # single SBUF staging tile + ONE dram write: 16 small writes can get
        # stuck behind collective DMA traffic and stall both the transpose
        # chain (buffer reuse) and the downstream AG trigger.
        ob = pool.tile([c.PF, c.KF, c.PT], out_dt, name=f"trs_{tag}",
                       tag="nrm_tr", bufs=2)
        for kf in range(c.KF):
            pt = psum.tile([c.PF, c.PT], BF16, name=f"ptr_{tag}_{kf}",
                           tag="ps_d", bufs=2)
            nc.tensor.transpose(pt, nrm[:, kf * c.PF : (kf + 1) * c.PF],
                                ident_pf[: c.PT, : c.PT])
            nc.scalar.copy(ob[:, kf, :], pt)
        nc.sync.dma_start(
            out=dst_dram.rearrange("(kf p) t -> p kf t", p=c.PF), in_=ob
        )

    # ================= phase 1: norm1 (per g) + split AG =================
    ph1 = tc.alloc_tile_pool(name="ph1", bufs=2)
    wab = ph1.tile([c.PT, c.D], FP32, tag="wab", bufs=1)
    nc.sync.dma_start(out=wab, in_=io["w_attn"].to_broadcast((c.PT, c.D)))
    for g in range(c.G):
        xt = resid.tile([c.PT, c.D], FP32, name=f"x_keep{g}", tag=f"x_keep{g}")
        nc.sync.dma_start(out=xt, in_=io["x_own"][g * c.PT : (g + 1) * c.PT, :])
        x_sb.append(xt)
        rmsnorm_to_dram(ph1, xt, wab, nrm_own[g], f"n1g{g}", out_dt=F8)
        nc.gpsimd.collective_compute(
            "AllGather", ALU.bypass, replica_groups=RG,
            ins=[nrm_own[g][:].opt()], outs=[ag1[g][:].opt()],
        )
    ph1.release()

    # ================= phase 2: QKV =================
    wo_pool = tc.alloc_tile_pool(name="wo_pool", bufs=1, side="right")
    qkv_w = tc.alloc_tile_pool(name="qkv_w", bufs=1)
    nrm_pool = tc.alloc_tile_pool(name="nrm_full", bufs=1)
    qkv_out = tc.alloc_tile_pool(name="qkv_out", bufs=1)

    # normed-own (feature-major) for the alpha-fold: data is ready right
    # after norm1, so both groups preload during the collective-init window
    # (emitted before the big phase-2 loads -- the sync queue is in-order).
    nrm_my = []
    for g in range(c.G):
        t = wo_pool.tile([c.PF, c.KF, c.PT], F8, name=f"nrm_my{g}",
                         tag=f"nrm_my{g}", bufs=1)
        nc.sync.dma_start(
            out=t, in_=nrm_own[g].rearrange("(kf p) t -> p kf t", p=c.PF)
        )
        nrm_my.append(t)

    masks = []
    for r in range(c.TPC):
        m = qkv_w.tile([c.PT, c.NCH], F8, name=f"mask{r}", tag=f"mask{r}")
        nc.vector.memset(m, 1.0)
        if r > 0:
            nc.vector.memset(m[:, : r * c.PT], 0.0)
        diag = m[:, r * c.PT : (r + 1) * c.PT]
        nc.vector.memset(diag, 0.0)
        nc.gpsimd.affine_select(
            out=diag, in_=diag, compare_op=ALU.is_gt, fill=1.0, base=0,
            pattern=[[-1, c.PT]], channel_multiplier=1,
        )
        masks.append(m)

    w_sb = {}
    for nm in ("wqT", "wkT", "wvT"):
        t = qkv_w.tile([c.PF, c.KF, c.HL * c.DH], F8, name=f"{nm}_sb", tag=nm)
        nc.sync.dma_start(out=t, in_=io[nm].rearrange("(kf p) m -> p kf m", p=c.PF))
        w_sb[nm] = t
    # full Wo resident in SBUF (f8, 4MB) -- no weight streaming during wo_part
    wo_sb = wo_pool.tile([c.PF, c.KF, c.D], F8, name="wo_sb", tag="wo_sb")
    nc.sync.dma_start(out=wo_sb, in_=io["woT"].rearrange("(kf p) m -> p kf m", p=c.PF))

    # full normed feature-major via DMA-transpose loads: one [PF, KF, S] tile
    # (kf-pairs adjacent along the free dim for DoubleRow rhs APs)
    nrm_sb = nrm_pool.tile([c.PF, c.KF, c.S], F8, name="nrm_sb", tag="nrm_sb")
    for g in range(c.G):
        src = ag1[g].rearrange("(r kf p) t -> kf p r t", r=c.NC, p=c.PF)
        for kf in range(c.KF):
            dst = nrm_sb[
                :, kf, g * c.NC * c.PT : (g + 1) * c.NC * c.PT
            ].rearrange("p (r t) -> p r t", r=c.NC)
            nc.sync.dma_start(out=dst, in_=src[kf])

    q_sb, k_sb = [], []
    for h in range(c.HL):
        q_sb.append(qkv_out.tile([c.DH, c.S], BF16, name=f"q_sb{h}", tag=f"q_sb{h}"))
        k_sb.append(qkv_out.tile([c.DH, c.S], BF16, name=f"k_sb{h}", tag=f"k_sb{h}"))
    # vT in f8, tile-pairs adjacent for DoubleRow AV lhsT
    vT_all = qkv_out.tile([c.PT, c.TT, c.HL * c.DH], F8, name="vT_all", tag="vT_all")
    MQ = max(1, (c.HL * c.DH) // c.PF)   # v-major tiles
    VP = (c.HL * c.DH) // MQ             # partition rows per v tile
    v_sb = [
        qkv_out.tile([VP, c.S], BF16, name=f"v_sb{m}", tag=f"v_sb{m}")
        for m in range(MQ)
    ]
    KP = c.KF // 2  # kf pairs (fp8 DoubleRow)

    def qkv_group(g):
        chunks = range(g * CPG, (g + 1) * CPG)
        # q, k per head; v in the same (weights-stationary) orientation
        jobs = []
        for h in range(c.HL):
            jobs.append((q_sb[h], "wqT", h * c.DH, c.DH))
            jobs.append((k_sb[h], "wkT", h * c.DH, c.DH))
        for m in range(MQ):
            jobs.append((v_sb[m], "wvT", m * VP, VP))
        for dst, wname, off, rows in jobs:
            pq = [
                psum.tile([rows, c.NCH], FP32, name=f"pq{g}{wname}{off}{i}",
                          tag="ps_mm", bufs=3)
                for i in range(CPG)
            ]
            for jf in range(KP):
                for i, nch in enumerate(chunks):
                    nc.tensor.matmul(
                        pq[i],
                        lhsT=w_sb[wname][:, 2 * jf : 2 * jf + 2, off : off + rows],
                        rhs=nrm_sb[:, 2 * jf : 2 * jf + 2,
                                   nch * c.NCH : (nch + 1) * c.NCH],
                        start=(jf == 0),
                        stop=(jf == KP - 1),
                        perf_mode=DR,
                    )
            for i, nch in enumerate(chunks):
                nc.scalar.copy(dst[:rows, nch * c.NCH : (nch + 1) * c.NCH], pq[i])
        # transpose v -> vT tiles for this g's tokens
        for t in range(g * c.NC, (g + 1) * c.NC):
            for m in range(MQ):
                ptv = psum.tile([c.PT, VP], BF16, name=f"ptv{t}{m}",
                                tag="ps_d", bufs=2)
                nc.tensor.transpose(
                    ptv, v_sb[m][:, t * c.PT : (t + 1) * c.PT],
                    ident_pf[:VP, :VP],
                )
                nc.scalar.copy(vT_all[:, t, m * VP : (m + 1) * VP], ptv)

    # ================= phase 3: attention =================
    attn_pool = tc.alloc_tile_pool(name="attn", bufs=2)

    def attn_chunk(nch):
        n_sk = c.TPC * (nch + 1)
        npair = n_sk // 2
        # both heads interleaved: while one head's exp chain runs, the other
        # head's score matmul keeps the PE busy. Accumulators for head 1 live
        # in the (attention-idle) ps_d/ps_sum slots.
        pav = [
            psum.tile([c.DH, c.NCH], FP32, name=f"pav{h}{nch}",
                      tag=("ps_av" if h == 0 else "ps_d"), bufs=1 if h == 0 else 2)
            for h in range(c.HL)
        ]
        psm = [
            psum.tile([c.DH, c.NCH], FP32, name=f"psm{h}{nch}",
                      tag="ps_sum", bufs=2)
            for h in range(c.HL)
        ]

        def score_exp(h, js):
            # exp'd scores for key-tile pair (2js, 2js+1), f8 plane-major
            pe_pair = attn_pool.tile([c.PT, 2, c.NCH], F8, name=f"pexp{h}{js}",
                                     tag="pexp", bufs=8)
            for pl in range(2):
                s = 2 * js + pl
                ps = psum.tile([c.PT, c.NCH], FP32, name=f"ps{h}{nch}{s}",
                               tag="ps_mm", bufs=3)
                nc.tensor.matmul(
                    ps,
                    lhsT=k_sb[h][:, s * c.PT : (s + 1) * c.PT],
                    rhs=q_sb[h][:, nch * c.NCH : (nch + 1) * c.NCH],
                    start=True, stop=True,
                )
                dst = pe_pair[:, pl, :]
                nc.scalar.activation(dst, ps, AF.Exp, scale=sc["c_exp"])
                if s >= c.TPC * nch:
                    nc.vector.tensor_mul(dst, dst, masks[s - c.TPC * nch])
            return pe_pair

        def sum_av(h, js, pe_pair):
            nc.tensor.matmul(
                psm[h], lhsT=ones_pair, rhs=pe_pair,
                start=(js == 0), stop=(js == npair - 1),
                perf_mode=DR,
            )
            nc.tensor.matmul(
                pav[h],
                lhsT=vT_all[:, 2 * js : 2 * js + 2, h * c.DH : (h + 1) * c.DH],
                rhs=pe_pair,
                start=(js == 0), stop=(js == npair - 1),
                perf_mode=DR,
            )

        LAG = 4
        pend = []
        for js in range(npair):
            for h in range(c.HL):
                pend.append((h, js, score_exp(h, js)))
                if len(pend) > LAG:
                    sum_av(*pend.pop(0))
        for it in pend:
            sum_av(*it)

        for h in range(c.HL):
            bc = attn_pool.tile([c.DH, c.NCH], FP32, name="bc", tag="bc")
            nc.vector.reciprocal(bc, psm[h])
            af = attn_pool.tile([c.DH, c.NCH], F8, name="af", tag="af")
            nc.vector.scalar_tensor_tensor(
                out=af, in0=pav[h], scalar=sc["sv"], in1=bc,
                op0=ALU.mult, op1=ALU.mult,
            )
            for t in range(c.TPC):
                tt = nch * c.TPC + t
                j, g = tt % c.NC, tt // c.NC
                nc.sync.dma_start(
                    out=attn_in[g][j * DSH + h * c.DH : j * DSH + (h + 1) * c.DH, :],
                    in_=af[:, t * c.PT : (t + 1) * c.PT],
                )

    # ================= per-group Wo machinery ============
    hpt = c.PF // c.DH  # heads per feature tile

    def wo_prep(g):
        # afull[:, kf, :] = a2a'd attention heads + alpha*normed (alpha-fold
        # done here on the DVE instead of a second Wo matmul chain); single
        # f8 tile so kf pairs sit adjacent for DoubleRow lhsT APs. The
        # strided load is split across 4 engine queues so the ~18us
        # single-queue pattern cost runs 4-way parallel.
        afull = wo_pool.tile([c.PF, c.KF, c.PT], F8, name=f"afull{g}",
                             tag="afull", bufs=2)
        src = attn_a2a[g].rearrange("(kf p) t -> p kf t", p=c.PF)
        for eng, lo, hi in ((nc.sync, 0, 6), (nc.scalar, 6, 12),
                            (nc.gpsimd, 12, 16)):
            eng.dma_start(out=afull[:, lo:hi, :], in_=src[:, lo:hi, :])
        for kf in range(c.KF):
            nc.vector.scalar_tensor_tensor(
                out=afull[:, kf, :], in0=nrm_my[g][:, kf, :],
                scalar=alpha_cols[:, kf : kf + 1],
                in1=afull[:, kf, :], op0=ALU.mult, op1=ALU.add,
            )
        x2 = resid.tile([c.PT, c.D], FP32, name=f"x2_keep{g}", tag=f"x2_keep{g}")
        x2_sb.append(x2)
        xsq = wo_pool.tile([c.PT, c.D], FP32, name=f"xsq_wo{g}", tag=f"xsq{g}",
                           bufs=1)
        return afull, x2, xsq

    def wo_part(g, pool, nds, st):
        afull, x2, xsq = st
        for nd in nds:
            po = psum.tile([c.PT, c.NCH], FP32, name=f"po{g}{nd}",
                           tag="ps_mm", bufs=3)
            for jf in range(c.KF // 2):
                nc.tensor.matmul(
                    po,
                    lhsT=afull[:, 2 * jf : 2 * jf + 2, :],
                    rhs=wo_sb[:, 2 * jf : 2 * jf + 2,
                              nd * c.NCH : (nd + 1) * c.NCH],
                    start=(jf == 0), stop=(jf == c.KF // 2 - 1),
                    perf_mode=DR,
                )
            cs = slice(nd * c.NCH, (nd + 1) * c.NCH)
            nc.vector.scalar_tensor_tensor(
                out=x2[:, cs], in0=po, scalar=sc["so"], in1=x_sb[g][:, cs],
                op0=ALU.mult, op1=ALU.add,
            )
            nc.vector.tensor_mul(xsq[:, cs], x2[:, cs], x2[:, cs])

    def norm2_ag(g, pool, st, wmb):
        _, x2, xsq = st
        rmsnorm_to_dram(pool, x2, wmb, nrm2_own[g], f"n2g{g}", xsq=xsq, out_dt=F8)
        nc.gpsimd.collective_compute(
            "AllGather", ALU.bypass, replica_groups=RG,
            ins=[nrm2_own[g][:].opt()], outs=[ag2[g][:].opt()],
        )

    # ---- QKV(g0) -> attn 0,1 -> A2A(g0) -> QKV(g1) -> attn 2,3 ----
    # attention chunks 0-1 only need g0 tokens' q/k/v, so they run while
    # QKV(g1) waits on its gathered activations; A2A(g0) fires early.
    qkv_group(0)
    attn_chunk(0)
    attn_chunk(1)
    nc.gpsimd.collective_compute(
        "AllToAll", ALU.bypass, replica_groups=RG,
        ins=[attn_in[0][:].opt()], outs=[attn_a2a[0][:].opt()],
    )
    st0 = wo_prep(0)  # loads/STTs overlap QKV(g1)+attn chunk 2
    qkv_group(1)
    attn_chunk(2)
    wo_part(0, attn_pool, range(0, c.ND // 2), st0)
    attn_chunk(3)
    nc.gpsimd.collective_compute(
        "AllToAll", ALU.bypass, replica_groups=RG,
        ins=[attn_in[1][:].opt()], outs=[attn_a2a[1][:].opt()],
    )
    wo_part(0, attn_pool, range(c.ND // 2, c.ND), st0)
    norm2_ag(0, attn_pool, st0, wmb)
    st1 = wo_prep(1)  # loads/STTs fire as soon as A2A(g1) lands

    attn_pool.release()
    qkv_out.release()
    nrm_pool.release()
    qkv_w.release()

    # ================= weights for MLP (prefetch, loaded ONCE) =============
    mlp_w = tc.alloc_tile_pool(name="mlp_w", bufs=1)
    wd_sb = mlp_w.tile([c.PF, c.MD, c.D], BF16)
    nc.sync.dma_start(out=wd_sb, in_=io["wdT"].rearrange("(kd p) m -> p kd m", p=c.PF))
    wg_sb = mlp_w.tile([c.PF, c.MD, c.KF, c.PF], F8)
    nc.sync.dma_start(
        out=wg_sb,
        in_=io["wgT"].rearrange("(kf p) (m f) -> p m kf f", p=c.PF, f=c.PF),
    )
    wu_sb = mlp_w.tile([c.PF, c.MD, c.KF, c.PF], F8)
    nc.sync.dma_start(
        out=wu_sb,
        in_=io["wuT"].rearrange("(kf p) (m f) -> p m kf f", p=c.PF, f=c.PF),
    )


    # ================= Wo(g=1) + norm2 + AG, then MLP + RS ============
    def mlp_group(g):
        # normed2 for both chunks of this group, feature-major via transpose;
        # one f8 tile per chunk so kf pairs sit adjacent for DoubleRow rhs.
        n2c = [
            mlp.tile([c.PF, c.KF, c.NCH], F8, name=f"n2c{g}{i}",
                     tag=f"n2c{i}", bufs=2)
            for i in range(CPG)
        ]
        src = ag2[g].rearrange("(r kf p) t -> kf p r t", r=c.NC, p=c.PF)
        for i in range(CPG):
            r0 = i * c.TPC
            for kf in range(c.KF):
                nc.sync.dma_start(
                    out=n2c[i][:, kf, :].rearrange("p (r t) -> p r t", r=c.TPC),
                    in_=src[kf][:, r0 : r0 + c.TPC],
                )
        h_sb = [[None] * c.MD for _ in range(CPG)]
        for m in range(c.MD):
            pg = [
                psum.tile([c.PF, c.NCH], FP32, name=f"pg{g}{m}{i}",
                          tag="ps_mm", bufs=3)
                for i in range(CPG)
            ]
            for jf in range(c.KF // 2):
                for i in range(CPG):
                    nc.tensor.matmul(
                        pg[i],
                        lhsT=wg_sb[:, m, 2 * jf : 2 * jf + 2, :],
                        rhs=n2c[i][:, 2 * jf : 2 * jf + 2, :],
                        start=(jf == 0), stop=(jf == c.KF // 2 - 1),
                        perf_mode=DR,
                    )
            sig = [None] * CPG
            for i in range(CPG):
                sig[i] = mlp.tile([c.PF, c.NCH], BF16, name=f"sig{g}{m}{i}",
                                  tag="sig", bufs=2)
                nc.scalar.activation(sig[i], pg[i], AF.Sigmoid, scale=sc["sg"])
            pu = [
                psum.tile([c.PF, c.NCH], FP32, name=f"pu{g}{m}{i}",
                          tag="ps_d", bufs=2)
                for i in range(CPG)
            ]
            for jf in range(c.KF // 2):
                for i in range(CPG):
                    nc.tensor.matmul(
                        pu[i],
                        lhsT=wu_sb[:, m, 2 * jf : 2 * jf + 2, :],
                        rhs=n2c[i][:, 2 * jf : 2 * jf + 2, :],
                        start=(jf == 0), stop=(jf == c.KF // 2 - 1),
                        perf_mode=DR,
                    )
            for i in range(CPG):
                gsw = mlp.tile([c.PF, c.NCH], BF16, name=f"gsw{g}{m}{i}",
                               tag="gsw", bufs=2)
                nc.vector.tensor_tensor(gsw, sig[i], pg[i], op=ALU.mult)
                ht = mlp.tile([c.PF, c.NCH], BF16, name=f"h{g}{m}{i}",
                              tag=f"h{m}_{i}", bufs=1)
                nc.vector.tensor_tensor(ht, gsw, pu[i], op=ALU.mult)
                h_sb[i][m] = ht

        # Wd: nd-major with RS per column group fired as soon as that group's
        # columns are complete.
        nd_part = {}
        for p, (st_nd, cnt) in enumerate(RS_SPLITS[g]):
            for nd in range(st_nd, st_nd + cnt):
                nd_part[nd] = (p, nd - st_nd, nd == st_nd + cnt - 1)

        def wd_tile(i, t, nd, pd):
            nch = g * CPG + i
            tt = nch * c.TPC + t
            row = (tt % c.NC) * c.PT
            for kd in range(c.MD):
                nc.tensor.matmul(
                    pd,
                    lhsT=h_sb[i][kd][:, t * c.PT : (t + 1) * c.PT],
                    rhs=wd_sb[:, kd, nd * c.NCH : (nd + 1) * c.NCH],
                    start=(kd == 0), stop=(kd == c.MD - 1),
                )
            mo = mlp.tile([c.PT, c.NCH], BF16, name=f"mo{g}{i}{t}{nd}", tag="mo",
                          bufs=6)
            nc.scalar.activation(mo, pd, AF.Copy, scale=sc["susd"])
            p, off, _ = nd_part[nd]
            nc.sync.dma_start(
                out=mlp_part[g][p][
                    row : row + c.PT, off * c.NCH : (off + 1) * c.NCH
                ],
                in_=mo,
            )

        for nd in range(c.ND):
            for i in range(CPG):
                for t in range(c.TPC):
                    pd = psum.tile([c.PT, c.NCH], FP32, name=f"pdl{g}{nd}{i}{t}",
                                   tag="ps_d", bufs=2)
                    wd_tile(i, t, nd, pd)
            p, _, is_last = nd_part[nd]
            if is_last:
                nc.gpsimd.collective_compute(
                    "ReduceScatter", ALU.add, replica_groups=RG,
                    ins=[mlp_part[g][p][:].opt()], outs=[rs_out[g][p][:].opt()],
                )

    ph4 = tc.alloc_tile_pool(name="ph4", bufs=2)
    wo_part(1, ph4, range(c.ND), st1)
    norm2_ag(1, ph4, st1, wmb)
    ph4.release()
    wo_pool.release()
    mlp = tc.alloc_tile_pool(name="mlp", bufs=2)

    # final residual for one group (fired right after its RS chunks)
    def finish_group(g):
        for p, (st_nd, cnt) in enumerate(RS_SPLITS[g]):
            cs = slice(st_nd * c.NCH, (st_nd + cnt) * c.NCH)
            rs_sb = mlp.tile([c.PT, cnt * c.NCH], BF16, name=f"rs_sb{g}{p}",
                             tag=f"rs_sb{p}")
            nc.sync.dma_start(out=rs_sb, in_=rs_out[g][p][:])
            ot = mlp.tile([c.PT, cnt * c.NCH], FP32, name=f"ot{g}{p}",
                          tag=f"ot{p}")
            nc.vector.tensor_tensor(ot, x2_sb[g][:, cs], rs_sb, op=ALU.add)
            nc.sync.dma_start(
                out=io["out_own"][g * c.PT : (g + 1) * c.PT, cs], in_=ot
            )

    mlp_group(0)
    finish_group(0)
    mlp_group(1)
    finish_group(1)

    mlp.release()
    mlp_w.release()
    resid.release()
    dram.release()
    psum.release()
    singles.release()



# ======================= host side =======================

def make_scales(sq, sk, sv, so, sg, su, sd, cfg: Cfg, alpha=None):
    return {
        "alpha": tuple(float(a) for a in np.asarray(alpha).reshape(-1))
        if alpha is not None else (0.0,) * cfg.H,
        "c_exp": float(sq) * float(sk) / math.sqrt(cfg.DH),
        "sv": float(sv),
        "so": float(so),
        "sg": float(sg),
        "susd": float(sg) * float(su) * float(sd),
        "eps": 1e-6,
    }


def prep_in_maps(cfg: Cfg, x, norm_attn_w, norm_mlp_w, Wq, Wk, Wv, Wo, Wg, Wu, Wd,
                 alpha):
    c = cfg
    bf = ml_dtypes.bfloat16
    f8 = ml_dtypes.float8_e4m3fn
    x0 = np.asarray(x, np.float32).reshape(c.S, c.D)
    woT = np.ascontiguousarray(np.asarray(Wo, np.float32).T).astype(f8)
    wa = np.asarray(norm_attn_w, np.float32).reshape(1, c.D)
    wm = np.asarray(norm_mlp_w, np.float32).reshape(1, c.D)
    in_maps = []
    for core in range(c.NC):
        hs = slice(core * c.HL * c.DH, (core + 1) * c.HL * c.DH)
        fs = slice(core * c.DFFL, (core + 1) * c.DFFL)
        rows = np.concatenate(
            [x0[(g * c.NC + core) * c.PT : (g * c.NC + core + 1) * c.PT]
             for g in range(c.G)]
        )
        in_maps.append({
            "x_own": np.ascontiguousarray(rows),
            "w_attn": wa.copy(),
            "w_mlp": wm.copy(),
            "wqT": np.ascontiguousarray(np.asarray(Wq, np.float32)[hs].T).astype(f8),
            "wkT": np.ascontiguousarray(np.asarray(Wk, np.float32)[hs].T).astype(f8),
            "wvT": np.ascontiguousarray(np.asarray(Wv, np.float32)[hs].T).astype(f8),
            "woT": woT.copy(),
            "wgT": np.ascontiguousarray(np.asarray(Wg, np.float32)[fs].T).astype(f8),
            "wuT": np.ascontiguousarray(np.asarray(Wu, np.float32)[fs].T).astype(f8),
            "wdT": np.ascontiguousarray(np.asarray(Wd, np.float32)[:, fs].T).astype(bf),
        })
    return in_maps


def assemble_out(cfg: Cfg, results):
    c = cfg
    out = np.zeros((c.S, c.D), np.float32)
    for core in range(c.NC):
        o = results[core]["out_own"]
        for g in range(c.G):
            out[(g * c.NC + core) * c.PT : (g * c.NC + core + 1) * c.PT] = o[
                g * c.PT : (g + 1) * c.PT
            ]
    return out.reshape(1, c.S, c.D)


def build_nc(cfg: Cfg, sc):
    nc = bacc.Bacc(
        "TRN2",
        target_bir_lowering=False,
        debug=False,
        enable_asserts=True,
        num_devices=cfg.NC,
    )
    io = declare_io(nc, cfg)
    with tile.TileContext(nc) as tc:
        build_block(tc, io, cfg, sc)
    nc.compile()
    return nc


_CACHE = {}


def kernel(x, norm_attn_w, norm_mlp_w, Wq, sq, Wk, sk, Wv, sv, Wo, so,
           Wg, sg, Wu, su, Wd, sd, alpha):
    cfg = FULL
    sc = make_scales(sq, sk, sv, so, sg, su, sd, cfg, alpha=alpha)
    key = tuple(sorted((k, v) for k, v in sc.items()))
    if key not in _CACHE:
        _CACHE[key] = build_nc(cfg, sc)
    nc = _CACHE[key]
    in_maps = prep_in_maps(
        cfg, x, norm_attn_w, norm_mlp_w, Wq, Wk, Wv, Wo, Wg, Wu, Wd, alpha
    )
    res = bass_utils.run_bass_kernel_spmd(
        nc, in_maps, core_ids=list(range(cfg.NC)),
        trace=bool(int(os.environ.get("KERNEL_TRACE", "0"))),
    )
    out = assemble_out(cfg, res.results)
    if res.exec_time_ns is not None:
        print(f"HW exec time: {res.exec_time_ns} ns", file=sys.stderr)
        kernel.last_exec_ns = res.exec_time_ns
    return out.astype(np.asarray(x).dtype)


kernel.last_exec_ns = None



# revision 78
# speedup vs baseline: 1.0551x; 1.0551x over previous
"""Trainium2 Bass kernel for nn_DiscreteTernaryBlock (8-core tensor-parallel).

Transformer block: rmsnorm -> QKV(ternary) -> causal attn (+alpha*x_heads)
-> Wo -> residual -> rmsnorm -> gated MLP (ternary) -> residual.

Sharding (all static SPMD; core c of 8):
  - tokens: 16 tiles of PT=128; core c OWNS global tiles {c, 8+c} (for norms,
    residuals, Wo, final output)
  - heads:  core c computes heads {2c, 2c+1} of attention (column-shard of
    Wq/Wk/Wv) over ALL tokens
  - mlp:    core c owns dff slice [1024c:1024c+1024] (column-shard Wg/Wu,
    row-shard Wd) over ALL tokens
Collectives: AG(normed) + A2A(x_heads) -> attn -> A2A(head outs) -> Wo ->
  AG(normed2) -> MLP -> 2x chunked ReduceScatter(partial mlp).

All matmuls run in bf16 (ternary weights are exact in bf16), fp32 PSUM
accumulation, fp32 residual path.
"""

import os
import sys
import math
from dataclasses import dataclass

import numpy as np

sys.path.insert(0, "/opt/trn_rl_repo")

import ml_dtypes

import concourse.bass as bass
import concourse.mybir as mybir
import concourse.tile as tile
from concourse import bacc
from concourse import bass_utils

BF16 = mybir.dt.bfloat16
FP32 = mybir.dt.float32
F8 = mybir.dt.float8e4
DR = mybir.MatmulPerfMode.DoubleRow
AF = mybir.ActivationFunctionType
ALU = mybir.AluOpType


@dataclass(frozen=True)
class Cfg:
    NC: int = 8     # cores
    G: int = 2      # token ownership groups (tiles per core)
    PT: int = 128   # token tile (partition dim for token-major tiles)
    DH: int = 128   # head dim
    HL: int = 2     # heads per core
    DFFL: int = 1024  # dff per core
    PF: int = 128   # feature tile (contraction partition dim)
    NCH: int = 512  # token chunk (matmul moving free dim)

    @property
    def D(self):
        return self.NC * self.HL * self.DH

    @property
    def S(self):
        return self.NC * self.G * self.PT

    @property
    def H(self):
        return self.NC * self.HL

    @property
    def DFF(self):
        return self.NC * self.DFFL

    @property
    def KF(self):
        return self.D // self.PF  # feature K tiles

    @property
    def TPC(self):
        return self.NCH // self.PT  # token tiles per chunk

    @property
    def NCHUNKS(self):
        return self.S // self.NCH

    @property
    def MD(self):
        return self.DFFL // self.PF  # dff tiles (per core)

    @property
    def ND(self):
        return self.D // self.NCH  # out-feature chunks

    @property
    def TT(self):
        return self.S // self.PT  # total token tiles


FULL = Cfg()
SMALL = Cfg(PT=16, DH=32, DFFL=128, NCH=64)


def declare_io(nc, cfg: Cfg):
    c = cfg
    io = {}

    def inp(name, shape, dt):
        io[name] = nc.dram_tensor(name, list(shape), dt, kind="ExternalInput").ap()

    inp("x_own", [c.G * c.PT, c.D], FP32)
    inp("w_attn", [1, c.D], FP32)
    inp("w_mlp", [1, c.D], FP32)
    inp("wqT", [c.D, c.HL * c.DH], F8)
    inp("wkT", [c.D, c.HL * c.DH], F8)
    inp("wvT", [c.D, c.HL * c.DH], F8)
    inp("woT", [c.D, c.D], F8)
    inp("wgT", [c.D, c.DFFL], F8)
    inp("wuT", [c.D, c.DFFL], F8)
    inp("wdT", [c.DFFL, c.D], BF16)
    io["out_own"] = nc.dram_tensor(
        "out_own", [c.G * c.PT, c.D], FP32, kind="ExternalOutput"
    ).ap()
    return io


def build_block(tc, io, cfg: Cfg, sc):
    """Emit the whole block program. sc: dict of baked scalar constants.

    Two pipelined token-group streams: for each ownership group g the chain
    norm1 -> AG -> qkv -> attn -> A2A -> Wo -> norm2 -> AG -> mlp -> RS runs
    offset so every collective overlaps the other group's compute.
    Gathered buffers are token-major; feature-major SBUF tiles are produced
    by DMA-transpose on load. alpha*x_heads is folded into a second Wo
    matmul (wo2T = alpha-scaled WoT) over the core's own normed tokens.
    """
    c = cfg
    nc = tc.nc
    RG = [list(range(c.NC))]
    DSH = c.D // c.NC       # feature rows per a2a shard (= HL*DH)
    CPG = c.NCHUNKS // c.G  # token chunks per ownership group

    # ---------------- pools ----------------
    singles = tc.alloc_tile_pool(name="singles", bufs=1)
    psum = tc.alloc_tile_pool(name="psum", bufs=1, space="PSUM")
    dram = tc.alloc_tile_pool(name="dram", bufs=1, space="DRAM")
    resid = tc.alloc_tile_pool(name="resid", bufs=1)

    # ---------------- dram scratch (split by ownership group g) ----------------
    HD = c.D // 2  # feature-half rows: AGs split in two so consumers start
    # on the first half while the second is still in flight
    nrm_own = [dram.tile([c.D, c.PT], F8, name=f"nrm_own{g}", tag=f"nrm_own{g}")
               for g in range(c.G)]
    ag1 = [dram.tile([c.NC * c.D, c.PT], F8, addr_space="Shared",
                     name=f"ag1_{g}", tag=f"ag1_{g}") for g in range(c.G)]
    attn_in = [dram.tile([c.D, c.PT], F8, name=f"attn_in{g}", tag=f"attn_in{g}")
               for g in range(c.G)]
    attn_a2a = [dram.tile([c.D, c.PT], F8, name=f"attn_a2a{g}", tag=f"attn_a2a{g}")
                for g in range(c.G)]
    nrm2_own = [dram.tile([c.D, c.PT], F8, name=f"nrm2_own{g}", tag=f"nrm2_own{g}")
                for g in range(c.G)]
    ag2 = [dram.tile([c.NC * c.D, c.PT], F8, addr_space="Shared",
                     name=f"ag2_{g}", tag=f"ag2_{g}") for g in range(c.G)]
    # RS chunking per group: [(start_nd, n_nds), ...]; the LAST group gets a
    # small final chunk so the un-overlapped tail RS is short.
    RS_SPLITS = [[(0, 2), (2, 2)], [(0, 1), (1, 1), (2, 1), (3, 1)]]
    mlp_part = [[dram.tile([c.NC * c.PT, cnt * c.NCH], BF16,
                           name=f"mlp_part{g}_{p}", tag=f"mlp_part{g}_{p}")
                 for p, (_, cnt) in enumerate(RS_SPLITS[g])] for g in range(c.G)]
    rs_out = [[dram.tile([c.PT, cnt * c.NCH], BF16,
                         name=f"rs_out{g}_{p}", tag=f"rs_out{g}_{p}")
               for p, (_, cnt) in enumerate(RS_SPLITS[g])] for g in range(c.G)]



    # ---------------- constants ----------------
    ident_pf = singles.tile([c.PF, c.PF], BF16)
    from concourse.masks import make_identity

    make_identity(nc, ident_pf)
    # [PT, 2, DH] of ones: the row-sum matmul uses all DH output partitions
    # so the softmax denominator lands replicated across partitions (no
    # gpsimd partition_broadcast / single-lane reciprocal needed).
    ones_pair = singles.tile([c.PT, 2, c.DH], F8)
    nc.vector.memset(ones_pair, 1.0)
    # alpha per feature-row, laid out as [PF, KF] (feature f -> alpha[f // DH])
    alpha_cols = singles.tile([c.PF, c.KF], FP32)
    for h in range(c.H):
        f0 = h * c.DH
        kf0, p0 = f0 // c.PF, f0 % c.PF
        nc.vector.memset(alpha_cols[p0 : p0 + c.DH, kf0 : kf0 + 1],
                         sc["alpha"][h])
    eps_sb = singles.tile([c.PT, 1], FP32)
    nc.vector.memset(eps_sb, sc["eps"])
    wmb = singles.tile([c.PT, c.D], FP32)
    nc.sync.dma_start(out=wmb, in_=io["w_mlp"].to_broadcast((c.PT, c.D)))

    x_sb = []   # per-group fp32 [PT, D] input rows (kept for residual)
    x2_sb = []  # per-group fp32 [PT, D] post-attention residual

    # ---------------- helpers ----------------
    def rmsnorm_to_dram(pool, src_sb, w_bc, dst_dram, tag, xsq=None, out_dt=BF16):
        """token-major src [PT, D] fp32 -> normed bf16 -> PE transpose ->
        feature-major dram [D, PT] (dtype out_dt)."""
        if xsq is None:
            xsq = pool.tile([c.PT, c.D], FP32, name=f"xsq_{tag}",
                            tag="nrm_xsq", bufs=1)
            nc.vector.tensor_mul(xsq, src_sb, src_sb)
        ssum = pool.tile([c.PT, 1], FP32, name=f"ssum_{tag}", tag="nrm_ssum")
        nc.vector.tensor_reduce(ssum, xsq, axis=mybir.AxisListType.X, op=ALU.add)
        nc.scalar.activation(ssum, ssum, AF.Sqrt, bias=eps_sb, scale=1.0 / c.D)
        nc.vector.reciprocal(ssum, ssum)
        nrm = pool.tile([c.PT, c.D], BF16, name=f"nrm_{tag}", tag="nrm_bf", bufs=1)
        nc.vector.scalar_tensor_tensor(
            out=nrm, in0=src_sb, scalar=ssum, in1=w_bc, op0=ALU.mult, op1=ALU.mult
        )
        # single SBUF staging tile + ONE dram write: 16 small writes can get
        # stuck behind collective DMA traffic and stall both the transpose
        # chain (buffer reuse) and the downstream AG trigger.
        ob = pool.tile([c.PF, c.KF, c.PT], out_dt, name=f"trs_{tag}",
                       tag="nrm_tr", bufs=2)
        for kf in range(c.KF):
            pt = psum.tile([c.PF, c.PT], BF16, name=f"ptr_{tag}_{kf}",
                           tag="ps_d", bufs=2)
            nc.tensor.transpose(pt, nrm[:, kf * c.PF : (kf + 1) * c.PF],
                                ident_pf[: c.PT, : c.PT])
            nc.scalar.copy(ob[:, kf, :], pt)
        nc.sync.dma_start(
            out=dst_dram.rearrange("(kf p) t -> p kf t", p=c.PF), in_=ob
        )

    # ================= phase 1: norm1 (per g) + split AG =================
    ph1 = tc.alloc_tile_pool(name="ph1", bufs=2)
    wab = ph1.tile([c.PT, c.D], FP32, tag="wab", bufs=1)
    nc.sync.dma_start(out=wab, in_=io["w_attn"].to_broadcast((c.PT, c.D)))
    for g in range(c.G):
        xt = resid.tile([c.PT, c.D], FP32, name=f"x_keep{g}", tag=f"x_keep{g}")
        nc.sync.dma_start(out=xt, in_=io["x_own"][g * c.PT : (g + 1) * c.PT, :])
        x_sb.append(xt)
        rmsnorm_to_dram(ph1, xt, wab, nrm_own[g], f"n1g{g}", out_dt=F8)
        nc.gpsimd.collective_compute(
            "AllGather", ALU.bypass, replica_groups=RG,
            ins=[nrm_own[g][:].opt()], outs=[ag1[g][:].opt()],
        )
    ph1.release()

    # ================= phase 2: QKV =================
    wo_pool = tc.alloc_tile_pool(name="wo_pool", bufs=1, side="right")
    qkv_w = tc.alloc_tile_pool(name="qkv_w", bufs=1)
    nrm_pool = tc.alloc_tile_pool(name="nrm_full", bufs=1)
    qkv_out = tc.alloc_tile_pool(name="qkv_out", bufs=1)

    # normed-own (feature-major) for the alpha-fold: data is ready right
    # after norm1, so both groups preload during the collective-init window
    # (emitted before the big phase-2 loads -- the sync queue is in-order).
    nrm_my = []
    for g in range(c.G):
        t = wo_pool.tile([c.PF, c.KF, c.PT], F8, name=f"nrm_my{g}",
                         tag=f"nrm_my{g}", bufs=1)
        nc.sync.dma_start(
            out=t, in_=nrm_own[g].rearrange("(kf p) t -> p kf t", p=c.PF)
        )
        nrm_my.append(t)

    masks = []
    for r in range(c.TPC):
        m = qkv_w.tile([c.PT, c.NCH], F8, name=f"mask{r}", tag=f"mask{r}")
        nc.vector.memset(m, 1.0)
        if r > 0:
            nc.vector.memset(m[:, : r * c.PT], 0.0)
        diag = m[:, r * c.PT : (r + 1) * c.PT]
        nc.vector.memset(diag, 0.0)
        nc.gpsimd.affine_select(
            out=diag, in_=diag, compare_op=ALU.is_gt, fill=1.0, base=0,
            pattern=[[-1, c.PT]], channel_multiplier=1,
        )
        masks.append(m)

    w_sb = {}
    for nm in ("wqT", "wkT", "wvT"):
        t = qkv_w.tile([c.PF, c.KF, c.HL * c.DH], F8, name=f"{nm}_sb", tag=nm)
        nc.sync.dma_start(out=t, in_=io[nm].rearrange("(kf p) m -> p kf m", p=c.PF))
        w_sb[nm] = t
    # full Wo resident in SBUF (f8, 4MB) -- no weight streaming during wo_part
    wo_sb = wo_pool.tile([c.PF, c.KF, c.D], F8, name="wo_sb", tag="wo_sb")
    nc.sync.dma_start(out=wo_sb, in_=io["woT"].rearrange("(kf p) m -> p kf m", p=c.PF))

    # full normed feature-major via DMA-transpose loads: one [PF, KF, S] tile
    # (kf-pairs adjacent along the free dim for DoubleRow rhs APs)
    nrm_sb = nrm_pool.tile([c.PF, c.KF, c.S], F8, name="nrm_sb", tag="nrm_sb")
    for g in range(c.G):
        src = ag1[g].rearrange("(r kf p) t -> kf p r t", r=c.NC, p=c.PF)
        for kf in range(c.KF):
            dst = nrm_sb[
                :, kf, g * c.NC * c.PT : (g + 1) * c.NC * c.PT
            ].rearrange("p (r t) -> p r t", r=c.NC)
            nc.sync.dma_start(out=dst, in_=src[kf])

    q_sb, k_sb = [], []
    for h in range(c.HL):
        q_sb.append(qkv_out.tile([c.DH, c.S], BF16, name=f"q_sb{h}", tag=f"q_sb{h}"))
        k_sb.append(qkv_out.tile([c.DH, c.S], BF16, name=f"k_sb{h}", tag=f"k_sb{h}"))
    # vT in f8, tile-pairs adjacent for DoubleRow AV lhsT
    vT_all = qkv_out.tile([c.PT, c.TT, c.HL * c.DH], F8, name="vT_all", tag="vT_all")
    MQ = max(1, (c.HL * c.DH) // c.PF)   # v-major tiles
    VP = (c.HL * c.DH) // MQ             # partition rows per v tile
    v_sb = [
        qkv_out.tile([VP, c.S], BF16, name=f"v_sb{m}", tag=f"v_sb{m}")
        for m in range(MQ)
    ]
    KP = c.KF // 2  # kf pairs (fp8 DoubleRow)

    def qkv_group(g):
        chunks = range(g * CPG, (g + 1) * CPG)
        # q, k per head; v in the same (weights-stationary) orientation
        jobs = []
        for h in range(c.HL):
            jobs.append((q_sb[h], "wqT", h * c.DH, c.DH))
            jobs.append((k_sb[h], "wkT", h * c.DH, c.DH))
        for m in range(MQ):
            jobs.append((v_sb[m], "wvT", m * VP, VP))
        for dst, wname, off, rows in jobs:
            pq = [
                psum.tile([rows, c.NCH], FP32, name=f"pq{g}{wname}{off}{i}",
                          tag="ps_mm", bufs=3)
                for i in range(CPG)
            ]
            for jf in range(KP):
                for i, nch in enumerate(chunks):
                    nc.tensor.matmul(
                        pq[i],
                        lhsT=w_sb[wname][:, 2 * jf : 2 * jf + 2, off : off + rows],
                        rhs=nrm_sb[:, 2 * jf : 2 * jf + 2,
                                   nch * c.NCH : (nch + 1) * c.NCH],
                        start=(jf == 0),
                        stop=(jf == KP - 1),
                        perf_mode=DR,
                    )
            for i, nch in enumerate(chunks):
                nc.scalar.copy(dst[:rows, nch * c.NCH : (nch + 1) * c.NCH], pq[i])
        # transpose v -> vT tiles for this g's tokens
        for t in range(g * c.NC, (g + 1) * c.NC):
            for m in range(MQ):
                ptv = psum.tile([c.PT, VP], BF16, name=f"ptv{t}{m}",
                                tag="ps_d", bufs=2)
                nc.tensor.transpose(
                    ptv, v_sb[m][:, t * c.PT : (t + 1) * c.PT],
                    ident_pf[:VP, :VP],
                )
                nc.scalar.copy(vT_all[:, t, m * VP : (m + 1) * VP], ptv)

    # ================= phase 3: attention =================
    attn_pool = tc.alloc_tile_pool(name="attn", bufs=2)

    def attn_chunk(nch):
        n_sk = c.TPC * (nch + 1)
        npair = n_sk // 2
        # both heads interleaved: while one head's exp chain runs, the other
        # head's score matmul keeps the PE busy. Accumulators for head 1 live
        # in the (attention-idle) ps_d/ps_sum slots.
        pav = [
            psum.tile([c.DH, c.NCH], FP32, name=f"pav{h}{nch}",
                      tag=("ps_av" if h == 0 else "ps_d"), bufs=1 if h == 0 else 2)
            for h in range(c.HL)
        ]
        psm = [
            psum.tile([c.DH, c.NCH], FP32, name=f"psm{h}{nch}",
                      tag="ps_sum", bufs=2)
            for h in range(c.HL)
        ]

        def score_exp(h, js):
            # exp'd scores for key-tile pair (2js, 2js+1), f8 plane-major.
            # q0: first non-fully-masked query column for this pair (causal)
            # -- columns below it are never read downstream.
            q0 = max(0, 2 * js - c.TPC * nch) * c.PT
            pe_pair = attn_pool.tile([c.PT, 2, c.NCH], F8, name=f"pexp{h}{js}",
                                     tag="pexp", bufs=8)
            for pl in range(2):
                s = 2 * js + pl
                ps = psum.tile([c.PT, c.NCH], FP32, name=f"ps{h}{nch}{s}",
                               tag="ps_mm", bufs=3)
                nc.tensor.matmul(
                    ps[:, q0:],
                    lhsT=k_sb[h][:, s * c.PT : (s + 1) * c.PT],
                    rhs=q_sb[h][:, nch * c.NCH + q0 : (nch + 1) * c.NCH],
                    start=True, stop=True,
                )
                dst = pe_pair[:, pl, q0:]
                nc.scalar.activation(dst, ps[:, q0:], AF.Exp, scale=sc["c_exp"])
                if s >= c.TPC * nch:
                    nc.vector.tensor_mul(
                        dst, dst, masks[s - c.TPC * nch][:, q0:]
                    )
            return pe_pair, q0

        def sum_av(h, js, pe_pair, q0):
            nc.tensor.matmul(
                psm[h][:, q0:], lhsT=ones_pair, rhs=pe_pair[:, :, q0:],
                start=(js == 0), stop=(js == npair - 1),
                perf_mode=DR,
            )
            nc.tensor.matmul(
                pav[h][:, q0:],
                lhsT=vT_all[:, 2 * js : 2 * js + 2, h * c.DH : (h + 1) * c.DH],
                rhs=pe_pair[:, :, q0:],
                start=(js == 0), stop=(js == npair - 1),
                perf_mode=DR,
            )

        LAG = 4
        pend = []
        for js in range(npair):
            for h in range(c.HL):
                pend.append((h, js) + score_exp(h, js))
                if len(pend) > LAG:
                    sum_av(*pend.pop(0))
        for it in pend:
            sum_av(*it)

        for h in range(c.HL):
            bc = attn_pool.tile([c.DH, c.NCH], FP32, name="bc", tag="bc")
            nc.vector.reciprocal(bc, psm[h])
            af = attn_pool.tile([c.DH, c.NCH], F8, name="af", tag="af")
            nc.vector.scalar_tensor_tensor(
                out=af, in0=pav[h], scalar=sc["sv"], in1=bc,
                op0=ALU.mult, op1=ALU.mult,
            )
            for t in range(c.TPC):
                tt = nch * c.TPC + t
                j, g = tt % c.NC, tt // c.NC
                nc.sync.dma_start(
                    out=attn_in[g][j * DSH + h * c.DH : j * DSH + (h + 1) * c.DH, :],
                    in_=af[:, t * c.PT : (t + 1) * c.PT],
                )

    # ================= per-group Wo machinery ============
    hpt = c.PF // c.DH  # heads per feature tile

    def wo_prep(g):
        # afull[:, kf, :] = a2a'd attention heads + alpha*normed (alpha-fold
        # done here on the DVE instead of a second Wo matmul chain); single
        # f8 tile so kf pairs sit adjacent for DoubleRow lhsT APs. The
        # strided load is split across 4 engine queues so the ~18us
        # single-queue pattern cost runs 4-way parallel.
        afull = wo_pool.tile([c.PF, c.KF, c.PT], F8, name=f"afull{g}",
                             tag="afull", bufs=2)
        src = attn_a2a[g].rearrange("(kf p) t -> p kf t", p=c.PF)
        for eng, lo, hi in ((nc.sync, 0, 6), (nc.scalar, 6, 12),
                            (nc.gpsimd, 12, 16)):
            eng.dma_start(out=afull[:, lo:hi, :], in_=src[:, lo:hi, :])
        for kf in range(c.KF):
            nc.vector.scalar_tensor_tensor(
                out=afull[:, kf, :], in0=nrm_my[g][:, kf, :],
                scalar=alpha_cols[:, kf : kf + 1],
                in1=afull[:, kf, :], op0=ALU.mult, op1=ALU.add,
            )
        x2 = resid.tile([c.PT, c.D], FP32, name=f"x2_keep{g}", tag=f"x2_keep{g}")
        x2_sb.append(x2)
        xsq = wo_pool.tile([c.PT, c.D], FP32, name=f"xsq_wo{g}", tag=f"xsq{g}",
                           bufs=1)
        return afull, x2, xsq

    def wo_part(g, pool, nds, st):
        afull, x2, xsq = st
        for nd in nds:
            po = psum.tile([c.PT, c.NCH], FP32, name=f"po{g}{nd}",
                           tag="ps_mm", bufs=3)
            for jf in range(c.KF // 2):
                nc.tensor.matmul(
                    po,
                    lhsT=afull[:, 2 * jf : 2 * jf + 2, :],
                    rhs=wo_sb[:, 2 * jf : 2 * jf + 2,
                              nd * c.NCH : (nd + 1) * c.NCH],
                    start=(jf == 0), stop=(jf == c.KF // 2 - 1),
                    perf_mode=DR,
                )
            cs = slice(nd * c.NCH, (nd + 1) * c.NCH)
            nc.vector.scalar_tensor_tensor(
                out=x2[:, cs], in0=po, scalar=sc["so"], in1=x_sb[g][:, cs],
                op0=ALU.mult, op1=ALU.add,
            )
            nc.vector.tensor_mul(xsq[:, cs], x2[:, cs], x2[:, cs])

    def norm2_ag(g, pool, st, wmb):
        _, x2, xsq = st
        rmsnorm_to_dram(pool, x2, wmb, nrm2_own[g], f"n2g{g}", xsq=xsq, out_dt=F8)
        nc.gpsimd.collective_compute(
            "AllGather", ALU.bypass, replica_groups=RG,
            ins=[nrm2_own[g][:].opt()], outs=[ag2[g][:].opt()],
        )

    # ---- QKV(g0) -> attn 0,1 -> A2A(g0) -> QKV(g1) -> attn 2,3 ----
    # attention chunks 0-1 only need g0 tokens' q/k/v, so they run while
    # QKV(g1) waits on its gathered activations; A2A(g0) fires early.
    qkv_group(0)
    attn_chunk(0)
    attn_chunk(1)
    nc.gpsimd.collective_compute(
        "AllToAll", ALU.bypass, replica_groups=RG,
        ins=[attn_in[0][:].opt()], outs=[attn_a2a[0][:].opt()],
    )
    st0 = wo_prep(0)  # loads/STTs overlap QKV(g1)+attn chunk 2
    qkv_group(1)
    attn_chunk(2)
    wo_part(0, attn_pool, range(0, c.ND // 2), st0)
    attn_chunk(3)
    nc.gpsimd.collective_compute(
        "AllToAll", ALU.bypass, replica_groups=RG,
        ins=[attn_in[1][:].opt()], outs=[attn_a2a[1][:].opt()],
    )
    wo_part(0, attn_pool, range(c.ND // 2, c.ND), st0)
    norm2_ag(0, attn_pool, st0, wmb)
    st1 = wo_prep(1)  # loads/STTs fire as soon as A2A(g1) lands

    attn_pool.release()
    qkv_out.release()
    nrm_pool.release()
    qkv_w.release()

    # ========= weights for MLP (prefetch, loaded ONCE, wg needed first) ====
    mlp_w = tc.alloc_tile_pool(name="mlp_w", bufs=1)
    wg_sb = mlp_w.tile([c.PF, c.MD, c.KF, c.PF], F8)
    nc.sync.dma_start(
        out=wg_sb,
        in_=io["wgT"].rearrange("(kf p) (m f) -> p m kf f", p=c.PF, f=c.PF),
    )
    wu_sb = mlp_w.tile([c.PF, c.MD, c.KF, c.PF], F8)
    nc.scalar.dma_start(
        out=wu_sb,
        in_=io["wuT"].rearrange("(kf p) (m f) -> p m kf f", p=c.PF, f=c.PF),
    )
    wd_sb = mlp_w.tile([c.PF, c.MD, c.D], BF16)
    nc.sync.dma_start(out=wd_sb, in_=io["wdT"].rearrange("(kd p) m -> p kd m", p=c.PF))


    # ================= Wo(g=1) + norm2 + AG, then MLP + RS ============
    def mlp_group(g):
        # normed2 for both chunks of this group, feature-major via transpose;
        # one f8 tile per chunk so kf pairs sit adjacent for DoubleRow rhs.
        n2c = [
            mlp.tile([c.PF, c.KF, c.NCH], F8, name=f"n2c{g}{i}",
                     tag=f"n2c{i}", bufs=2)
            for i in range(CPG)
        ]
        src = ag2[g].rearrange("(r kf p) t -> kf p r t", r=c.NC, p=c.PF)
        for i in range(CPG):
            r0 = i * c.TPC
            for kf in range(c.KF):
                nc.sync.dma_start(
                    out=n2c[i][:, kf, :].rearrange("p (r t) -> p r t", r=c.TPC),
                    in_=src[kf][:, r0 : r0 + c.TPC],
                )
        h_sb = [[None] * c.MD for _ in range(CPG)]
        for m in range(c.MD):
            pg = [
                psum.tile([c.PF, c.NCH], FP32, name=f"pg{g}{m}{i}",
                          tag="ps_mm", bufs=3)
                for i in range(CPG)
            ]
            for jf in range(c.KF // 2):
                for i in range(CPG):
                    nc.tensor.matmul(
                        pg[i],
                        lhsT=wg_sb[:, m, 2 * jf : 2 * jf + 2, :],
                        rhs=n2c[i][:, 2 * jf : 2 * jf + 2, :],
                        start=(jf == 0), stop=(jf == c.KF // 2 - 1),
                        perf_mode=DR,
                    )
            sig = [None] * CPG
            for i in range(CPG):
                sig[i] = mlp.tile([c.PF, c.NCH], BF16, name=f"sig{g}{m}{i}",
                                  tag="sig", bufs=2)
                nc.scalar.activation(sig[i], pg[i], AF.Sigmoid, scale=sc["sg"])
            pu = [
                psum.tile([c.PF, c.NCH], FP32, name=f"pu{g}{m}{i}",
                          tag="ps_d", bufs=2)
                for i in range(CPG)
            ]
            for jf in range(c.KF // 2):
                for i in range(CPG):
                    nc.tensor.matmul(
                        pu[i],
                        lhsT=wu_sb[:, m, 2 * jf : 2 * jf + 2, :],
                        rhs=n2c[i][:, 2 * jf : 2 * jf + 2, :],
                        start=(jf == 0), stop=(jf == c.KF // 2 - 1),
                        perf_mode=DR,
                    )
            for i in range(CPG):
                gsw = mlp.tile([c.PF, c.NCH], BF16, name=f"gsw{g}{m}{i}",
                               tag="gsw", bufs=2)
                nc.vector.tensor_tensor(gsw, sig[i], pg[i], op=ALU.mult)
                ht = mlp.tile([c.PF, c.NCH], BF16, name=f"h{g}{m}{i}",
                              tag=f"h{m}_{i}", bufs=1)
                nc.vector.tensor_tensor(ht, gsw, pu[i], op=ALU.mult)
                h_sb[i][m] = ht

        # Wd: nd-major with RS per column group fired as soon as that group's
        # columns are complete.
        nd_part = {}
        for p, (st_nd, cnt) in enumerate(RS_SPLITS[g]):
            for nd in range(st_nd, st_nd + cnt):
                nd_part[nd] = (p, nd - st_nd, nd == st_nd + cnt - 1)

        def wd_tile(i, t, nd, pd):
            nch = g * CPG + i
            tt = nch * c.TPC + t
            row = (tt % c.NC) * c.PT
            for kd in range(c.MD):
                nc.tensor.matmul(
                    pd,
                    lhsT=h_sb[i][kd][:, t * c.PT : (t + 1) * c.PT],
                    rhs=wd_sb[:, kd, nd * c.NCH : (nd + 1) * c.NCH],
                    start=(kd == 0), stop=(kd == c.MD - 1),
                )
            mo = mlp.tile([c.PT, c.NCH], BF16, name=f"mo{g}{i}{t}{nd}", tag="mo",
                          bufs=6)
            nc.scalar.activation(mo, pd, AF.Copy, scale=sc["susd"])
            p, off, _ = nd_part[nd]
            nc.sync.dma_start(
                out=mlp_part[g][p][
                    row : row + c.PT, off * c.NCH : (off + 1) * c.NCH
                ],
                in_=mo,
            )

        for nd in range(c.ND):
            for i in range(CPG):
                for t in range(c.TPC):
                    pd = psum.tile([c.PT, c.NCH], FP32, name=f"pdl{g}{nd}{i}{t}",
                                   tag="ps_d", bufs=2)
                    wd_tile(i, t, nd, pd)
            p, _, is_last = nd_part[nd]
            if is_last:
                nc.gpsimd.collective_compute(
                    "ReduceScatter", ALU.add, replica_groups=RG,
                    ins=[mlp_part[g][p][:].opt()], outs=[rs_out[g][p][:].opt()],
                )

    ph4 = tc.alloc_tile_pool(name="ph4", bufs=2)
    wo_part(1, ph4, range(c.ND), st1)
    norm2_ag(1, ph4, st1, wmb)
    ph4.release()
    wo_pool.release()
    mlp = tc.alloc_tile_pool(name="mlp", bufs=2)

    # final residual for one group (fired right after its RS chunks)
    def finish_group(g):
        for p, (st_nd, cnt) in enumerate(RS_SPLITS[g]):
            cs = slice(st_nd * c.NCH, (st_nd + cnt) * c.NCH)
            rs_sb = mlp.tile([c.PT, cnt * c.NCH], BF16, name=f"rs_sb{g}{p}",
                             tag=f"rs_sb{p}")
            nc.sync.dma_start(out=rs_sb, in_=rs_out[g][p][:])
            ot = mlp.tile([c.PT, cnt * c.NCH], FP32, name=f"ot{g}{p}",
                          tag=f"ot{p}")
            nc.vector.tensor_tensor(ot, x2_sb[g][:, cs], rs_sb, op=ALU.add)
            nc.sync.dma_start(
                out=io["out_own"][g * c.PT : (g + 1) * c.PT, cs], in_=ot
            )

    mlp_group(0)
    finish_group(0)
    mlp_group(1)
    finish_group(1)

    mlp.release()
    mlp_w.release()
    resid.release()
    dram.release()
    psum.release()
    singles.release()



# ======================= host side =======================

def make_scales(sq, sk, sv, so, sg, su, sd, cfg: Cfg, alpha=None):
    return {
        "alpha": tuple(float(a) for a in np.asarray(alpha).reshape(-1))
        if alpha is not None else (0.0,) * cfg.H,
        "c_exp": float(sq) * float(sk) / math.sqrt(cfg.DH),
        "sv": float(sv),
        "so": float(so),
        "sg": float(sg),
        "susd": float(sg) * float(su) * float(sd),
        "eps": 1e-6,
    }


def prep_in_maps(cfg: Cfg, x, norm_attn_w, norm_mlp_w, Wq, Wk, Wv, Wo, Wg, Wu, Wd,
                 alpha):
    c = cfg
    bf = ml_dtypes.bfloat16
    f8 = ml_dtypes.float8_e4m3fn
    x0 = np.asarray(x, np.float32).reshape(c.S, c.D)
    woT = np.ascontiguousarray(np.asarray(Wo, np.float32).T).astype(f8)
    wa = np.asarray(norm_attn_w, np.float32).reshape(1, c.D)
    wm = np.asarray(norm_mlp_w, np.float32).reshape(1, c.D)
    in_maps = []
    for core in range(c.NC):
        hs = slice(core * c.HL * c.DH, (core + 1) * c.HL * c.DH)
        fs = slice(core * c.DFFL, (core + 1) * c.DFFL)
        rows = np.concatenate(
            [x0[(g * c.NC + core) * c.PT : (g * c.NC + core + 1) * c.PT]
             for g in range(c.G)]
        )
        in_maps.append({
            "x_own": np.ascontiguousarray(rows),
            "w_attn": wa.copy(),
            "w_mlp": wm.copy(),
            "wqT": np.ascontiguousarray(np.asarray(Wq, np.float32)[hs].T).astype(f8),
            "wkT": np.ascontiguousarray(np.asarray(Wk, np.float32)[hs].T).astype(f8),
            "wvT": np.ascontiguousarray(np.asarray(Wv, np.float32)[hs].T).astype(f8),
            "woT": woT.copy(),
            "wgT": np.ascontiguousarray(np.asarray(Wg, np.float32)[fs].T).astype(f8),
            "wuT": np.ascontiguousarray(np.asarray(Wu, np.float32)[fs].T).astype(f8),
            "wdT": np.ascontiguousarray(np.asarray(Wd, np.float32)[:, fs].T).astype(bf),
        })
    return in_maps


def assemble_out(cfg: Cfg, results):
    c = cfg
    out = np.zeros((c.S, c.D), np.float32)
    for core in range(c.NC):
        o = results[core]["out_own"]
        for g in range(c.G):
            out[(g * c.NC + core) * c.PT : (g * c.NC + core + 1) * c.PT] = o[
                g * c.PT : (g + 1) * c.PT
            ]
    return out.reshape(1, c.S, c.D)


def build_nc(cfg: Cfg, sc):
    nc = bacc.Bacc(
        "TRN2",
        target_bir_lowering=False,
        debug=False,
        enable_asserts=True,
        num_devices=cfg.NC,
    )
    io = declare_io(nc, cfg)
    with tile.TileContext(nc) as tc:
        build_block(tc, io, cfg, sc)
    nc.compile()
    return nc


_CACHE = {}


def kernel(x, norm_attn_w, norm_mlp_w, Wq, sq, Wk, sk, Wv, sv, Wo, so,
           Wg, sg, Wu, su, Wd, sd, alpha):
    cfg = FULL
    sc = make_scales(sq, sk, sv, so, sg, su, sd, cfg, alpha=alpha)
    key = tuple(sorted((k, v) for k, v in sc.items()))
    if key not in _CACHE:
        _CACHE[key] = build_nc(cfg, sc)
    nc = _CACHE[key]
    in_maps = prep_in_maps(
        cfg, x, norm_attn_w, norm_mlp_w, Wq, Wk, Wv, Wo, Wg, Wu, Wd, alpha
    )
    res = bass_utils.run_bass_kernel_spmd(
        nc, in_maps, core_ids=list(range(cfg.NC)),
        trace=bool(int(os.environ.get("KERNEL_TRACE", "0"))),
    )
    out = assemble_out(cfg, res.results)
    if res.exec_time_ns is not None:
        print(f"HW exec time: {res.exec_time_ns} ns", file=sys.stderr)
        kernel.last_exec_ns = res.exec_time_ns
    return out.astype(np.asarray(x).dtype)


kernel.last_exec_ns = None

